# revision 1
# baseline (speedup 1.0000x reference)
"""T5-style MultiHeadAttention (relative position bias) on 8 Trainium2 cores.

Sharding: core c = (b, g) with b = c // 4 (batch), g = c % 4 (head group of 4
heads).  Each core computes q/k/v projections for its 4 heads, attention with
the relative-position bias, and a partial output projection (rows of Wo for
its heads).  Host sums the 4 partials per batch element.

Key layout choices (per core):
  - x is passed transposed: xT [1024, 2048] so projections contract over
    partitions directly.
  - Q_t, K_t stored as [d, seq] (d on partitions); scores computed
    *transposed* as S_t[k, q] = K_t^T-stationary matmul, so that exp(S_t) can
    be used directly as the stationary operand of the attn@V matmul (which
    contracts over k = partitions).
  - Softmax denominator Z[q] = sum_k exp(S_t[k, q]) falls out of the attn@V
    matmul for free via a ones-column appended to V (M=65 per head).
  - No max-subtraction: scores are O(50) at most, exp stays finite in fp32
    and bf16.
  - Relative-position bias applied multiplicatively after exp:
    exp(s + bias) = exp(s) * exp(bias).  bias[k, q] = v_h[k - q + 2047] is
    Toeplitz, so exp(bias) tiles are overlapping windows of a single
    [128, 3968] sliding table per head, precomputed on host:
      T_h[p, i] = exp(v_h[p + 3967 - i])
    and the tile for (k0 = kc*128, q0 = qb*512) is T_h[:, base:base+512] with
    base = 1920 - k0 + q0.
  - Matmuls run in float32r (full PE rate at N>=256); the attention
    probabilities / V use bf16 (configurable) for the 2x DVE multiply mode.
"""

import numpy as np
from contextlib import ExitStack

import concourse.bass as bass
import concourse.tile as tile
from concourse import bacc, mybir
from concourse.bass_utils import run_bass_kernel_spmd

# ---------------------------------------------------------------- constants
B, S, D_MODEL, N_HEADS, D_KV = 2, 2048, 1024, 16, 64
NUM_BUCKETS, MAX_DIST = 32, 128
N_CORES = 8
HPC = N_HEADS // (N_CORES // B)  # heads per core = 4
DH = HPC * D_KV                  # 256 d-cols per core
TBL = 3968                       # exp-bias sliding table width
QB = 512                         # q block (free dim of score tiles)
KC = 128                         # k chunk (partition dim of score tiles)

F32 = mybir.dt.float32
F32R = mybir.dt.float32r
BF16 = mybir.dt.bfloat16
AF = mybir.ActivationFunctionType

# attention-probability dtype: BF16 (fast DVE 2x) or F32 (accurate, 1x DVE)
ATT_DT = BF16

_cache = {}


# ------------------------------------------------------------- host helpers
def _rel_bucket(d):
    """Bucket of relative position d = k - q (bidirectional T5), numpy fp32
    mirror of the jax reference."""
    nb = NUM_BUCKETS // 2
    n = -d
    ret = (n < 0).astype(np.int32) * nb
    n = np.abs(n)
    max_exact = nb // 2
    is_small = n < max_exact
    nf = np.maximum(n, 1).astype(np.float32)
    val = (
        np.log(nf / np.float32(max_exact))
        / np.float32(np.log(MAX_DIST / max_exact))
        * np.float32(nb - max_exact)
    ).astype(np.int32) + max_exact
    val = np.minimum(val, nb - 1)
    return ret + np.where(is_small, n, val)


def _expbias_tables(rel_emb):
    """[N_HEADS, 128, TBL] exp-bias sliding tables (float32)."""
    d = np.arange(-(S - 1), S)  # k - q in [-2047, 2047]
    buck = _rel_bucket(d)  # [4095]
    vals = rel_emb[buck, :].astype(np.float32)  # [4095, H]
    idx = np.arange(KC)[:, None] + (TBL - 1) - np.arange(TBL)[None, :]
    t = np.exp(vals[idx, :])  # [128, TBL, H]
    return np.ascontiguousarray(np.transpose(t, (2, 0, 1)))


# ------------------------------------------------------------- kernel body
def mha_body(tc, outs, ins, ckpt=None):
    nc = tc.nc
    ctx = ExitStack()
    xt_d = ins["xt"].bitcast(F32R)        # [1024, 2048]
    wq_d = ins["wq"].bitcast(F32R)        # [1024, 256]
    wk_d = ins["wk"].bitcast(F32R)        # [1024, 256]
    wv_d = ins["wv"].bitcast(F32R)        # [1024, 256]
    wo_d = ins["wo"].bitcast(F32R)        # [256, 1024]
    eb_d = ins["expb"]      # [HPC, 128, TBL] ATT_DT
    out_d = outs["out"]     # [2048, 1024] f32

    att_np = ATT_DT
    DKN = D_MODEL // 128    # 8 contraction chunks
    NQ = S // QB            # 4 q blocks
    NK = S // KC            # 16 k chunks

    def r(ap):  # operands already float32r
        return ap

    with ctx:
        const = ctx.enter_context(tc.tile_pool(name="const", bufs=1))

        # ---- persistent SBUF tensors
        qt = [const.tile([128, S], F32R, tag=f"qt{i}", name=f"qt{i}") for i in range(2)]
        kt = [const.tile([128, S], F32R, tag=f"kt{i}", name=f"kt{i}") for i in range(2)]
        # V with a ones column per head: [k, 4*65]; bf16 (AV stationary)
        vsb = [const.tile([128, HPC * 65], att_np, tag=f"v{i}", name=f"v{i}") for i in range(NK)]
        # normalized attention outputs, head-pairs stacked on partitions
        ust = [const.tile([128, S], F32R, tag=f"ust{i}", name=f"ust{i}") for i in range(2)]
        wo = [const.tile([128, D_MODEL], F32R, tag=f"wo{i}", name=f"wo{i}") for i in range(2)]
        ebp = ctx.enter_context(tc.tile_pool(name="expb", bufs=2))

        for i in range(2):
            nc.sync.dma_start(out=wo[i], in_=wo_d[i * 128:(i + 1) * 128, :])
        # prefetch head-pair 0's exp-bias tables before phase 1 so the
        # attention pipeline never stalls the PE at the phase boundary
        ebs = {}
        for j in range(2):
            eb = ebp.tile([128, TBL], att_np, tag=f"eb{j}", name=f"eb0_{j}")
            nc.sync.dma_start(out=eb, in_=eb_d[j])
            ebs[(0, j)] = eb

        # ---- flat pools (no scoped release: pool-release barriers idle the
        # PE >3.4us at phase boundaries and drop the HAM clock to 1.2GHz)
        wpool = ctx.enter_context(tc.tile_pool(name="wqkv", bufs=1))
        xtp = ctx.enter_context(tc.tile_pool(name="xts", bufs=6))
        xtvp = ctx.enter_context(tc.tile_pool(name="xtv", bufs=6))
        esp = ctx.enter_context(tc.tile_pool(name="es", bufs=4))
        esbp = ctx.enter_context(tc.tile_pool(name="esb", bufs=4))
        rzp = ctx.enter_context(tc.tile_pool(name="rz", bufs=2))
        outp = ctx.enter_context(tc.tile_pool(name="outsb", bufs=3))
        # one PSUM pool, 4 tags x 2 bufs of [128,512]f32 = 8 banks, shared by
        # all phases (no psum pool release barriers)
        pp = ctx.enter_context(tc.tile_pool(name="pp", bufs=2, space="PSUM"))

        # ================= phase 1: projections =================
        wq = [wpool.tile([128, DH], F32R, tag=f"wq{i}", name=f"wq{i}") for i in range(DKN)]
        wk = [wpool.tile([128, DH], F32R, tag=f"wk{i}", name=f"wk{i}") for i in range(DKN)]
        wv = [wpool.tile([128, DH], F32R, tag=f"wv{i}", name=f"wv{i}") for i in range(DKN)]
        for i in range(DKN):
            nc.sync.dma_start(out=wq[i], in_=wq_d[i * 128:(i + 1) * 128, :])
            nc.sync.dma_start(out=wk[i], in_=wk_d[i * 128:(i + 1) * 128, :])
            nc.sync.dma_start(out=wv[i], in_=wv_d[i * 128:(i + 1) * 128, :])

        # QK pass: xT streamed once; all 4 projections accumulate per block
        for qb in range(NQ):
            pq = [pp.tile([128, QB], F32, tag=t, name=f"pq{m}_{qb}")
                  for m, t in ((0, "a"), (1, "b"))]
            pk = [pp.tile([128, QB], F32, tag=t, name=f"pk{m}_{qb}")
                  for m, t in ((0, "c"), (1, "d"))]
            for dk in range(DKN):
                xtt = xtp.tile([128, QB], F32R, tag="xts", name=f"xt_{qb}_{dk}")
                nc.sync.dma_start(
                    out=xtt,
                    in_=xt_d[dk * 128:(dk + 1) * 128, qb * QB:(qb + 1) * QB])
                for m in range(2):
                    nc.tensor.matmul(
                        pq[m], wq[dk][:, m * 128:(m + 1) * 128], xtt,
                        start=(dk == 0), stop=(dk == DKN - 1))
                    nc.tensor.matmul(
                        pk[m], wk[dk][:, m * 128:(m + 1) * 128], xtt,
                        start=(dk == 0), stop=(dk == DKN - 1))
            for m in range(2):
                nc.scalar.copy(out=qt[m][:, qb * QB:(qb + 1) * QB], in_=pq[m])
                nc.scalar.copy(out=kt[m][:, qb * QB:(qb + 1) * QB], in_=pk[m])

        # V pass: xT re-streamed as [128,128] stationary tiles
        for kc in range(NK):
            pv = pp.tile([128, DH], F32, tag="abcd"[kc % 4], name=f"pv{kc}")
            for dk in range(DKN):
                xtt = xtvp.tile([128, 128], F32R, tag="xtv",
                                name=f"xtv_{kc}_{dk}")
                nc.sync.dma_start(
                    out=xtt,
                    in_=xt_d[dk * 128:(dk + 1) * 128, kc * 128:(kc + 1) * 128])
                nc.tensor.matmul(pv, xtt, wv[dk],
                                 start=(dk == 0), stop=(dk == DKN - 1))
            v3 = vsb[kc].rearrange("p (h c) -> p h c", h=HPC)
            nc.scalar.copy(
                out=v3[:, :, 0:64],
                in_=pv.rearrange("p (h c) -> p h c", h=HPC))
            nc.vector.memset(v3[:, :, 64:65], 1.0)

        # ================= phase 2+3: attention =================
        # Heads in pairs (2hp, 2hp+1): score matmuls use disjoint PE row
        # groups (rows 0-63 / 64-127) so weight loads + streaming overlap.
        mulc = 0  # round-robin DVE/GpSimd mult offload
        for hp in range(2):
            if hp == 0:
                # prefetch pair 1's tables during pair 0's compute
                for j in range(2):
                    eb = ebp.tile([128, TBL], att_np, tag=f"eb{j}",
                                  name=f"eb1_{j}")
                    nc.sync.dma_start(out=eb, in_=eb_d[2 + j])
                    ebs[(1, j)] = eb
            for qb in range(NQ):
                pus = [pp.tile([65, QB], F32, tag=t, name=f"pu{j}_{hp}_{qb}")
                       for j, t in ((0, "c"), (1, "d"))]
                for kc in range(NK):
                    base = (TBL - S) - kc * 128 + qb * QB
                    pss, ess, esbs = [], [], []
                    for j in range(2):
                        prow = slice(j * 64, j * 64 + 64)
                        ps = pp.tile([128, QB], F32, tag="ab"[j],
                                     name=f"ps{j}_{kc}")
                        nc.tensor.matmul(
                            ps,
                            kt[hp][prow, kc * 128:(kc + 1) * 128],
                            qt[hp][prow, qb * QB:(qb + 1) * QB],
                            start=True, stop=True)
                        pss.append(ps)
                    for j in range(2):
                        es = esp.tile([128, QB], att_np, tag=f"es{j}",
                                      name=f"es{j}_{kc}")
                        nc.scalar.activation(out=es, in_=pss[j], func=AF.Exp)
                        ess.append(es)
                    for j in range(2):
                        esb = esbp.tile([128, QB], att_np, tag=f"esb{j}",
                                        name=f"esb{j}_{kc}")
                        eng = nc.gpsimd if (mulc % 3 == 2) else nc.vector
                        eng.tensor_mul(esb, ess[j],
                                       ebs[(hp, j)][:, base:base + QB])
                        mulc += 1
                        esbs.append(esb)
                    for j in range(2):
                        h = hp * 2 + j
                        nc.tensor.matmul(
                            pus[j], vsb[kc][:, h * 65:(h + 1) * 65], esbs[j],
                            start=(kc == 0), stop=(kc == NK - 1))
                # normalize U[d, q] / Z[q]; Z = row 64 of pu
                for j in range(2):
                    rz = rzp.tile([1, QB], F32, tag=f"rz{j}", name=f"rz{j}_{qb}")
                    nc.vector.reciprocal(out=rz, in_=pus[j][64:65, :])
                    rzb = rzp.tile([64, QB], F32, tag=f"rzb{j}",
                                   name=f"rzb{j}_{qb}")
                    nc.gpsimd.partition_broadcast(rzb, rz, channels=64)
                    if j == 0:
                        nc.vector.tensor_mul(
                            ust[hp][0:64, qb * QB:(qb + 1) * QB],
                            pus[j][0:64, :], rzb)
                    else:
                        # DVE lanes are partition-locked; write via a [64,512]
                        # staging tile then DMA to rows 64-127
                        stg = rzp.tile([64, QB], F32R, tag="stg",
                                       name=f"stg{hp}_{qb}")
                        nc.vector.tensor_mul(stg, pus[j][0:64, :], rzb)
                        nc.sync.dma_start(
                            out=ust[hp][64:128, qb * QB:(qb + 1) * QB],
                            in_=stg)

        # ================= phase 4: output projection =================
        for qc in range(S // 128):
            ob = outp.tile([128, D_MODEL], F32, tag="ob", name=f"ob{qc}")
            for e in range(2):
                po = pp.tile([128, 512], F32, tag="ab"[e], name=f"po{qc}_{e}")
                for i in range(2):
                    nc.tensor.matmul(
                        po,
                        ust[i][:, qc * 128:(qc + 1) * 128],
                        wo[i][:, e * 512:(e + 1) * 512],
                        start=(i == 0), stop=(i == 1))
                nc.vector.tensor_copy(out=ob[:, e * 512:(e + 1) * 512], in_=po)
            nc.sync.dma_start(out=out_d[qc * 128:(qc + 1) * 128, :], in_=ob)


# ------------------------------------------------------------- build + run
def _build():
    if "nc" in _cache:
        return _cache["nc"]
    nc = bacc.Bacc("TRN2", target_bir_lowering=False, debug=False)
    att_np_dt = mybir.dt.np(ATT_DT)
    ins = {
        "xt": nc.dram_tensor("xt", [D_MODEL, S], F32R, kind="ExternalInput").ap(),
        "wq": nc.dram_tensor("wq", [D_MODEL, DH], F32R, kind="ExternalInput").ap(),
        "wk": nc.dram_tensor("wk", [D_MODEL, DH], F32R, kind="ExternalInput").ap(),
        "wv": nc.dram_tensor("wv", [D_MODEL, DH], F32R, kind="ExternalInput").ap(),
        "wo": nc.dram_tensor("wo", [DH, D_MODEL], F32R, kind="ExternalInput").ap(),
        "expb": nc.dram_tensor("expb", [HPC, KC, TBL], ATT_DT,
                               kind="ExternalInput").ap(),
    }
    outs = {
        "out": nc.dram_tensor("out", [S, D_MODEL], F32, kind="ExternalOutput").ap(),
    }
    with tile.TileContext(nc) as tc:
        mha_body(tc, outs, ins)
    nc.compile()
    _cache["nc"] = nc
    return nc


TRACE = False
LAST = {}


def kernel(inputs, Wq, Wk, Wv, Wo, rel_emb):
    inputs = np.asarray(inputs, dtype=np.float32)
    Wq = np.asarray(Wq, dtype=np.float32)
    Wk = np.asarray(Wk, dtype=np.float32)
    Wv = np.asarray(Wv, dtype=np.float32)
    Wo = np.asarray(Wo, dtype=np.float32)
    rel_emb = np.asarray(rel_emb, dtype=np.float32)

    nc = _build()
    att_np_dt = mybir.dt.np(ATT_DT)

    ebt = _expbias_tables(rel_emb)  # [16, 128, TBL] f32
    in_maps = []
    for c in range(N_CORES):
        b, g = c // (N_CORES // B), c % (N_CORES // B)
        hs = slice(g * DH, (g + 1) * DH)
        in_maps.append({
            "xt": np.ascontiguousarray(inputs[b].T),
            "wq": np.ascontiguousarray(Wq[:, hs]),
            "wk": np.ascontiguousarray(Wk[:, hs]),
            "wv": np.ascontiguousarray(Wv[:, hs]),
            "wo": np.ascontiguousarray(Wo[hs, :]),
            "expb": np.ascontiguousarray(
                ebt[g * HPC:(g + 1) * HPC]).astype(att_np_dt),
        })

    res = run_bass_kernel_spmd(
        nc, in_maps, core_ids=list(range(N_CORES)), trace=TRACE)
    LAST["res"] = res

    out = np.zeros((B, S, D_MODEL), dtype=np.float64)
    for c in range(N_CORES):
        b = c // (N_CORES // B)
        out[b] += res.results[c]["out"].astype(np.float64)
    return out.astype(np.float32)



# revision 2
# speedup vs baseline: 205.6990x; 205.6990x over previous
"""T5-style MultiHeadAttention (relative position bias) on 8 Trainium2 cores.

Sharding: core c = (b, g) with b = c // 4 (batch), g = c % 4 (head group of 4
heads).  Each core computes q/k/v projections for its 4 heads, attention with
the relative-position bias, and a partial output projection (rows of Wo for
its heads).  Host sums the 4 partials per batch element.

v2 restructure (vs the phase-sequential v1):
  - V projection is folded into the K pass using the streamed xt tiles as
    *stationary* operands (pv[k, dh] accumulated over d_model chunks), so xt
    is never re-streamed as 128 tiny [128,128] DMAs (the v1 V pass serialized
    ~72us of DMA-queue sequencer time).
  - Two passes: pass 1 = K+V over all seq; pass 2 = per q-block: Q projection
    -> attention (both head pairs) -> output projection for the previous
    q-block (software-pipelined by one block).  The Activation engine starts
    exp work ~30us into the kernel instead of ~110us, and the output
    projection overlaps the next block's attention.
  - Scores for a head pair land in one [128,1024] PSUM tile (2 banks), so
    exp runs as 128 x [128,1024] Activation ops instead of 256 x [128,512]
    (halves per-instruction overhead on the busiest engine).
  - exp-bias multiply: one DVE op per (qb,kc) over the pair-packed table
    (bf16 SBUF operands keep DVE in its fast mode).
  - PSUM->SBUF copies run on Pool/Activation depending on which is idle in
    that pass (Act does pass-1 copies, Pool does pass-2 copies).
  - DMA issue order is first-needed-first: wk/wv interleaved with the first
    q-block's xt tiles, wq/wo/exp-bias tables after.

Numerics (unchanged from v1): no max-subtraction (scores are O(50), exp is
finite in fp32/bf16); softmax denominator via a ones-column appended to V
(M=65 per head); bias applied multiplicatively after exp, exp(bias) tiles are
windows of a per-head [128, 3968] sliding table built on host.
"""

import numpy as np
from contextlib import ExitStack

import concourse.bass as bass
import concourse.tile as tile
from concourse import bacc, mybir
from concourse.bass_utils import run_bass_kernel_spmd

# ---------------------------------------------------------------- constants
B, S, D_MODEL, N_HEADS, D_KV = 2, 2048, 1024, 16, 64
NUM_BUCKETS, MAX_DIST = 32, 128
N_CORES = 8
HPC = N_HEADS // (N_CORES // B)  # heads per core = 4
DH = HPC * D_KV                  # 256 d-cols per core
TBL = 3968                       # exp-bias sliding table width
QB = 512                         # q block (free dim of score tiles)
KC = 128                         # k chunk (partition dim of score tiles)
NQ = S // QB                     # 4 q blocks
NK = S // KC                     # 16 k chunks
DKN = D_MODEL // 128             # 8 contraction chunks

F32 = mybir.dt.float32
F32R = mybir.dt.float32r
BF16 = mybir.dt.bfloat16
AF = mybir.ActivationFunctionType

# attention-probability dtype: BF16 (fast DVE 2x) or F32 (accurate, 1x DVE)
ATT_DT = BF16

_cache = {}


# ------------------------------------------------------------- host helpers
def _rel_bucket(d):
    """Bucket of relative position d = k - q (bidirectional T5), numpy fp32
    mirror of the jax reference."""
    nb = NUM_BUCKETS // 2
    n = -d
    ret = (n < 0).astype(np.int32) * nb
    n = np.abs(n)
    max_exact = nb // 2
    is_small = n < max_exact
    nf = np.maximum(n, 1).astype(np.float32)
    val = (
        np.log(nf / np.float32(max_exact))
        / np.float32(np.log(MAX_DIST / max_exact))
        * np.float32(nb - max_exact)
    ).astype(np.int32) + max_exact
    val = np.minimum(val, nb - 1)
    return ret + np.where(is_small, n, val)


def _expbias_tables(rel_emb):
    """[N_HEADS, 128, TBL] exp-bias sliding tables (float32).

    T_h[p, i] = exp(v_h[p + 3967 - i]) where v_h[j] = rel_emb[bucket(j-2047)].
    Built as sliding windows over exp(v_h) reversed (no 8M-element exp)."""
    d = np.arange(-(S - 1), S)              # k - q in [-2047, 2047]
    buck = _rel_bucket(d)                   # [4095]
    e = np.exp(rel_emb[buck, :].astype(np.float32))  # [4095, H]
    r = e[::-1]                             # r[k] = e[4094-k]
    # sw[k, h, i] = r[k+i, h];  T_h[p, i] = e[p+3967-i, h] = r[127-p+i, h]
    sw = np.lib.stride_tricks.sliding_window_view(r, TBL, axis=0)  # [128,H,TBL]
    return np.ascontiguousarray(np.transpose(sw[::-1], (1, 0, 2)))


def _pack_expb(ebt_4, att_np_dt):
    """Pair-pack a core's 4 head tables [4,128,TBL] -> [2,128,2*TBL]:
    pair hp holds heads (2hp, 2hp+1) side by side along the free dim."""
    out = np.empty((2, KC, 2 * TBL), dtype=att_np_dt)
    for hp in range(2):
        out[hp, :, :TBL] = ebt_4[2 * hp]
        out[hp, :, TBL:] = ebt_4[2 * hp + 1]
    return out


# ------------------------------------------------------------- kernel body
def mha_body(tc, outs, ins, ckpt=None, dump=None):
    nc = tc.nc
    ctx = ExitStack()
    xt_d = ins["xt"].bitcast(F32R)        # [1024, 2048]
    wq_d = ins["wq"].bitcast(F32R)        # [1024, 256]
    wk_d = ins["wk"].bitcast(F32R)        # [1024, 256]
    wv_d = ins["wv"].bitcast(F32R)        # [1024, 256]
    wo_d = ins["wo"].bitcast(F32R)        # [256, 1024]
    eb_d = ins["expb"]                    # [2, 128, 2*TBL] ATT_DT
    out_d = outs["out"]                   # [2048, 1024] f32

    att_np = ATT_DT

    with ctx:
        const = ctx.enter_context(tc.tile_pool(name="const", bufs=1))

        # ---- persistent SBUF tensors
        qt = [const.tile([128, S], F32R, tag=f"qt{i}", name=f"qt{i}") for i in range(2)]
        kt = [const.tile([128, S], F32R, tag=f"kt{i}", name=f"kt{i}") for i in range(2)]
        # V with a ones column per head: [k, 4*65]; bf16 (AV stationary)
        vsb = [const.tile([128, HPC * 65], att_np, tag=f"v{i}", name=f"v{i}")
               for i in range(NK)]
        # normalized attention outputs, head-pairs stacked on partitions
        ust = [const.tile([128, S], F32R, tag=f"ust{i}", name=f"ust{i}") for i in range(2)]
        wo = [const.tile([128, D_MODEL], F32R, tag=f"wo{i}", name=f"wo{i}") for i in range(2)]
        # pair-packed exp-bias tables, head j of pair hp at cols [j*TBL, (j+1)*TBL)
        ebp = [const.tile([128, 2 * TBL], att_np, tag=f"eb{i}", name=f"eb{i}")
               for i in range(2)]

        wpool = ctx.enter_context(tc.tile_pool(name="wqkv", bufs=1))
        wq = [wpool.tile([128, DH], F32R, tag=f"wq{i}", name=f"wq{i}") for i in range(DKN)]
        wk = [wpool.tile([128, DH], F32R, tag=f"wk{i}", name=f"wk{i}") for i in range(DKN)]
        wv = [wpool.tile([128, DH], F32R, tag=f"wv{i}", name=f"wv{i}") for i in range(DKN)]

        # ---- flat working pools (no scoped release barriers)
        xtp = ctx.enter_context(tc.tile_pool(name="xts", bufs=4))
        esp = ctx.enter_context(tc.tile_pool(name="es", bufs=3))
        esbp = ctx.enter_context(tc.tile_pool(name="esb", bufs=3))
        rzp = ctx.enter_context(tc.tile_pool(name="rz", bufs=2))
        outp = ctx.enter_context(tc.tile_pool(name="outsb", bufs=2))
        # PSUM: tag q [128,512]x2 (2 banks) for pk/pq/po, tag s [128,1024]x2
        # (4 banks) for pv/ps_pair, tag u [65,512]x2 (2 banks) for pus
        pp = ctx.enter_context(tc.tile_pool(name="pp", bufs=2, space="PSUM"))

        # ones columns of V (disjoint from the V copies below)
        for kc in range(NK):
            v3 = vsb[kc].rearrange("p (h c) -> p h c", h=HPC)
            nc.vector.memset(v3[:, :, 64:65], 1.0)

        # ================= pass 1: Q + K + V =================
        # One xt stream feeds all three projections per q block: Q/K use xt
        # as the moving operand, V uses its [128,128] column slices as the
        # stationary operand (pv[c] accumulates [k, dh] over d_model chunks).
        # All 8 PSUM banks are live per block: pq(u x2) pk(q x2) pv(a2+b2).
        # DMA issue: weights interleaved with qb=0's xt tiles, first-needed-
        # first; xt split across the SP and (idle) Pool queues; eb/wo late.
        for qb in range(NQ):
            pk = [pp.tile([128, QB], F32, tag="q", name=f"pk{m}_{qb}")
                  for m in range(2)]
            pq = [pp.tile([128, QB], F32, tag="u", name=f"pq{m}_{qb}")
                  for m in range(2)]
            # 4 k-chunk V accumulators [128,256], one PSUM bank each (two
            # accumulation chains may not share a bank's zero region)
            pv = [pp.tile([128, DH], F32, tag=t, name=f"pv{c}_{qb}")
                  for c, t in enumerate(("a", "a", "b", "b"))]
            for dk in range(DKN):
                if qb == 0:
                    nc.sync.dma_start(
                        out=wk[dk], in_=wk_d[dk * 128:(dk + 1) * 128, :])
                    nc.sync.dma_start(
                        out=wv[dk], in_=wv_d[dk * 128:(dk + 1) * 128, :])
                    nc.gpsimd.dma_start(
                        out=wq[dk], in_=wq_d[dk * 128:(dk + 1) * 128, :])
                # xt stream alternates SP / Pool queues (Pool is idle;
                # halves the serialized DMA SEQ time)
                eng = nc.gpsimd if dk % 2 else nc.sync
                xtt = xtp.tile([128, QB], F32R, tag="xt",
                               name=f"xt_p1_{qb}_{dk}")
                eng.dma_start(
                    out=xtt,
                    in_=xt_d[dk * 128:(dk + 1) * 128, qb * QB:(qb + 1) * QB])
                for m in range(2):
                    nc.tensor.matmul(
                        pk[m], wk[dk][:, m * 128:(m + 1) * 128], xtt,
                        start=(dk == 0), stop=(dk == DKN - 1))
                for m in range(2):
                    nc.tensor.matmul(
                        pq[m], wq[dk][:, m * 128:(m + 1) * 128], xtt,
                        start=(dk == 0), stop=(dk == DKN - 1))
                for c in range(4):
                    nc.tensor.matmul(
                        pv[c], xtt[:, c * 128:(c + 1) * 128], wv[dk],
                        start=(dk == 0), stop=(dk == DKN - 1))
            if qb == 1:
                # wo/eb transfers land in the pass-1 tail / early pass 2
                for i in range(2):
                    nc.sync.dma_start(out=wo[i], in_=wo_d[i * 128:(i + 1) * 128, :])
                    nc.gpsimd.dma_start(out=ebp[i], in_=eb_d[i])
            # PSUM->SBUF copies on DVE (idle during pass 1; Act only exps)
            for m in range(2):
                nc.vector.tensor_copy(
                    out=kt[m][:, qb * QB:(qb + 1) * QB], in_=pk[m])
            for m in range(2):
                nc.vector.tensor_copy(
                    out=qt[m][:, qb * QB:(qb + 1) * QB], in_=pq[m])
            for c in range(4):
                v3 = vsb[4 * qb + c].rearrange("p (h c) -> p h c", h=HPC)
                nc.vector.tensor_copy(
                    out=v3[:, :, 0:64],
                    in_=pv[c].rearrange("p (h c) -> p h c", h=HPC))

        # ================= pass 2: per q block =================
        def emit_outproj_unit(qc, e):
            po = pp.tile([128, 512], F32, tag="q", name=f"po{qc}_{e}")
            for i in range(2):
                nc.tensor.matmul(
                    po,
                    ust[i][:, qc * 128:(qc + 1) * 128],
                    wo[i][:, e * 512:(e + 1) * 512],
                    start=(i == 0), stop=(i == 1))
            ob = outp.tile([128, 512], F32, tag="ob", name=f"ob{qc}_{e}")
            nc.vector.tensor_copy(out=ob, in_=po)
            nc.sync.dma_start(
                out=out_d[qc * 128:(qc + 1) * 128, e * 512:(e + 1) * 512],
                in_=ob)

        # software pipeline: out-proj(qb-1) units fill PE slack inside this
        # block's k loops (attention is Activation-bound).
        for qb in range(NQ):
            for hp in range(2):
                pus = [pp.tile([65, QB], F32, tag="u", name=f"pu{j}_{hp}_{qb}")
                       for j in range(2)]
                for kc in range(NK):
                    base = (TBL - S) - kc * 128 + qb * QB
                    ps = [pp.tile([128, QB], F32, tag=t,
                                  name=f"ps{t}_{hp}_{qb}_{kc}")
                          for t in ("a", "b")]
                    for j in range(2):
                        prow = slice(j * 64, j * 64 + 64)
                        nc.tensor.matmul(
                            ps[j],
                            kt[hp][prow, kc * 128:(kc + 1) * 128],
                            qt[hp][prow, qb * QB:(qb + 1) * QB],
                            start=True, stop=True)
                    es = esp.tile([128, 2 * QB], att_np, tag="es",
                                  name=f"es_{hp}_{qb}_{kc}")
                    for j in range(2):
                        nc.scalar.activation(
                            out=es[:, j * QB:(j + 1) * QB],
                            in_=ps[j], func=AF.Exp)
                    esb = esbp.tile([128, 2 * QB], att_np, tag="esb",
                                    name=f"esb_{hp}_{qb}_{kc}")
                    for j in range(2):
                        nc.vector.tensor_mul(
                            esb[:, j * QB:(j + 1) * QB],
                            es[:, j * QB:(j + 1) * QB],
                            ebp[hp][:, j * TBL + base:j * TBL + base + QB])
                    for j in range(2):
                        h = hp * 2 + j
                        nc.tensor.matmul(
                            pus[j], vsb[kc][:, h * 65:(h + 1) * 65],
                            esb[:, j * QB:(j + 1) * QB],
                            start=(kc == 0), stop=(kc == NK - 1))
                    # interleaved fill work for the PE: 8 out-proj units of
                    # block qb-1 spread over this block's 32 kc slots
                    if qb >= 1 and kc % 4 == 3:
                        u = hp * 4 + kc // 4
                        emit_outproj_unit((qb - 1) * 4 + u // 2, u % 2)
                # normalize U[d, q] / Z[q]; Z = row 64 of pus
                for j in range(2):
                    rz = rzp.tile([1, QB], F32, tag=f"rz{j}",
                                  name=f"rz{j}_{hp}_{qb}")
                    nc.vector.reciprocal(out=rz, in_=pus[j][64:65, :])
                    rzb = rzp.tile([64, QB], F32, tag=f"rzb{j}",
                                   name=f"rzb{j}_{hp}_{qb}")
                    nc.gpsimd.partition_broadcast(rzb, rz, channels=64)
                    if j == 0:
                        nc.vector.tensor_mul(
                            ust[hp][0:64, qb * QB:(qb + 1) * QB],
                            pus[j][0:64, :], rzb)
                    else:
                        # DVE lanes are partition-locked; write via a [64,512]
                        # staging tile then DMA to rows 64-127
                        stg = rzp.tile([64, QB], F32R, tag="stg",
                                       name=f"stg{hp}_{qb}")
                        nc.vector.tensor_mul(stg, pus[j][0:64, :], rzb)
                        nc.sync.dma_start(
                            out=ust[hp][64:128, qb * QB:(qb + 1) * QB],
                            in_=stg)

        for qc in range((NQ - 1) * 4, NQ * 4):
            for e in range(2):
                emit_outproj_unit(qc, e)

        if dump is not None:
            _, dbg = dump
            for i in range(2):
                nc.sync.dma_start(out=dbg["d_kt"][i], in_=kt[i].bitcast(F32))
                nc.sync.dma_start(out=dbg["d_qt"][i], in_=qt[i].bitcast(F32))
                nc.sync.dma_start(out=dbg["d_ust"][i], in_=ust[i].bitcast(F32))
            for kc in range(NK):
                stg = outp.tile([128, HPC * 65], F32, tag="ob", name=f"dv{kc}")
                nc.vector.tensor_copy(out=stg, in_=vsb[kc])
                nc.sync.dma_start(out=dbg["d_vsb"][kc], in_=stg)


# ------------------------------------------------------------- build + run
def _build():
    if "nc" in _cache:
        return _cache["nc"]
    nc = bacc.Bacc("TRN2", target_bir_lowering=False, debug=False)
    ins = {
        "xt": nc.dram_tensor("xt", [D_MODEL, S], F32R, kind="ExternalInput").ap(),
        "wq": nc.dram_tensor("wq", [D_MODEL, DH], F32R, kind="ExternalInput").ap(),
        "wk": nc.dram_tensor("wk", [D_MODEL, DH], F32R, kind="ExternalInput").ap(),
        "wv": nc.dram_tensor("wv", [D_MODEL, DH], F32R, kind="ExternalInput").ap(),
        "wo": nc.dram_tensor("wo", [DH, D_MODEL], F32R, kind="ExternalInput").ap(),
        "expb": nc.dram_tensor("expb", [2, KC, 2 * TBL], ATT_DT,
                               kind="ExternalInput").ap(),
    }
    outs = {
        "out": nc.dram_tensor("out", [S, D_MODEL], F32, kind="ExternalOutput").ap(),
    }
    with tile.TileContext(nc) as tc:
        mha_body(tc, outs, ins)
    nc.compile()
    _cache["nc"] = nc
    return nc


TRACE = False
LAST = {}


def _in_maps(inputs, Wq, Wk, Wv, Wo, rel_emb):
    att_np_dt = mybir.dt.np(ATT_DT)
    ebt = _expbias_tables(rel_emb)  # [16, 128, TBL] f32
    maps = []
    for c in range(N_CORES):
        b, g = c // (N_CORES // B), c % (N_CORES // B)
        hs = slice(g * DH, (g + 1) * DH)
        maps.append({
            "xt": np.ascontiguousarray(inputs[b].T),
            "wq": np.ascontiguousarray(Wq[:, hs]),
            "wk": np.ascontiguousarray(Wk[:, hs]),
            "wv": np.ascontiguousarray(Wv[:, hs]),
            "wo": np.ascontiguousarray(Wo[hs, :]),
            "expb": _pack_expb(ebt[g * HPC:(g + 1) * HPC], att_np_dt),
        })
    return maps


def kernel(inputs, Wq, Wk, Wv, Wo, rel_emb):
    inputs = np.asarray(inputs, dtype=np.float32)
    Wq = np.asarray(Wq, dtype=np.float32)
    Wk = np.asarray(Wk, dtype=np.float32)
    Wv = np.asarray(Wv, dtype=np.float32)
    Wo = np.asarray(Wo, dtype=np.float32)
    rel_emb = np.asarray(rel_emb, dtype=np.float32)

    nc = _build()
    in_maps = _in_maps(inputs, Wq, Wk, Wv, Wo, rel_emb)
    res = run_bass_kernel_spmd(
        nc, in_maps, core_ids=list(range(N_CORES)), trace=TRACE)
    LAST["res"] = res

    out = np.zeros((B, S, D_MODEL), dtype=np.float32)
    for c in range(N_CORES):
        b = c // (N_CORES // B)
        out[b] += res.results[c]["out"]
    return out


# revision 11
# speedup vs baseline: 1658.6405x; 8.0634x over previous
"""T5-style MultiHeadAttention (relative position bias) on 8 Trainium2 cores.

Sharding: core c = (b, g) with b = c // 4 (batch), g = c % 4 (head group of 4
heads).  Each core computes q/k/v projections for its 4 heads, attention with
the relative-position bias, and a partial output projection (rows of Wo for
its heads).  Host sums the 4 partials per batch element.

v2 restructure (vs the phase-sequential v1):
  - V projection is folded into the K pass using the streamed xt tiles as
    *stationary* operands (pv[k, dh] accumulated over d_model chunks), so xt
    is never re-streamed as 128 tiny [128,128] DMAs (the v1 V pass serialized
    ~72us of DMA-queue sequencer time).
  - Two passes: pass 1 = K+V over all seq; pass 2 = per q-block: Q projection
    -> attention (both head pairs) -> output projection for the previous
    q-block (software-pipelined by one block).  The Activation engine starts
    exp work ~30us into the kernel instead of ~110us, and the output
    projection overlaps the next block's attention.
  - Scores for a head pair land in one [128,1024] PSUM tile (2 banks), so
    exp runs as 128 x [128,1024] Activation ops instead of 256 x [128,512]
    (halves per-instruction overhead on the busiest engine).
  - exp-bias multiply: one DVE op per (qb,kc) over the pair-packed table
    (bf16 SBUF operands keep DVE in its fast mode).
  - PSUM->SBUF copies run on Pool/Activation depending on which is idle in
    that pass (Act does pass-1 copies, Pool does pass-2 copies).
  - DMA issue order is first-needed-first: wk/wv interleaved with the first
    q-block's xt tiles, wq/wo/exp-bias tables after.

Numerics (unchanged from v1): no max-subtraction (scores are O(50), exp is
finite in fp32/bf16); softmax denominator via a ones-column appended to V
(M=65 per head); bias applied multiplicatively after exp, exp(bias) tiles are
windows of a per-head [128, 3968] sliding table built on host.
"""

import numpy as np
from contextlib import ExitStack

import concourse.bass as bass
import concourse.tile as tile
from concourse import bacc, mybir
from concourse.bass_utils import run_bass_kernel_spmd

# ---------------------------------------------------------------- constants
B, S, D_MODEL, N_HEADS, D_KV = 2, 2048, 1024, 16, 64
NUM_BUCKETS, MAX_DIST = 32, 128
N_CORES = 8
HPC = N_HEADS // (N_CORES // B)  # heads per core = 4
DH = HPC * D_KV                  # 256 d-cols per core
TBL = 3968                       # exp-bias sliding table width
QB = 512                         # q block (free dim of score tiles)
KC = 128                         # k chunk (partition dim of score tiles)
NQ = S // QB                     # 4 q blocks
NK = S // KC                     # 16 k chunks
DKN = D_MODEL // 128             # 8 contraction chunks

F32 = mybir.dt.float32
F32R = mybir.dt.float32r
F16 = mybir.dt.float16
BF16 = mybir.dt.bfloat16
AF = mybir.ActivationFunctionType

# attention-probability dtype: BF16 (fast DVE 2x) or F32 (accurate, 1x DVE)
ATT_DT = BF16

_cache = {}


# ------------------------------------------------------------- host helpers
def _rel_bucket(d):
    """Bucket of relative position d = k - q (bidirectional T5), numpy fp32
    mirror of the jax reference."""
    nb = NUM_BUCKETS // 2
    n = -d
    ret = (n < 0).astype(np.int32) * nb
    n = np.abs(n)
    max_exact = nb // 2
    is_small = n < max_exact
    nf = np.maximum(n, 1).astype(np.float32)
    val = (
        np.log(nf / np.float32(max_exact))
        / np.float32(np.log(MAX_DIST / max_exact))
        * np.float32(nb - max_exact)
    ).astype(np.int32) + max_exact
    val = np.minimum(val, nb - 1)
    return ret + np.where(is_small, n, val)


def _expbias_tables(rel_emb):
    """[N_HEADS, 128, TBL] exp-bias sliding tables (float32).

    T_h[p, i] = exp(v_h[p + 3967 - i]) where v_h[j] = rel_emb[bucket(j-2047)].
    Built as sliding windows over exp(v_h) reversed (no 8M-element exp)."""
    d = np.arange(-(S - 1), S)              # k - q in [-2047, 2047]
    buck = _rel_bucket(d)                   # [4095]
    e = np.exp(rel_emb[buck, :].astype(np.float32))  # [4095, H]
    r = e[::-1]                             # r[k] = e[4094-k]
    # sw[k, h, i] = r[k+i, h];  T_h[p, i] = e[p+3967-i, h] = r[127-p+i, h]
    sw = np.lib.stride_tricks.sliding_window_view(r, TBL, axis=0)  # [128,H,TBL]
    return np.ascontiguousarray(np.transpose(sw[::-1], (1, 0, 2)))


def _pack_expb(ebt_4, att_np_dt):
    """Pair-pack a core's 4 head tables [4,128,TBL] -> [2,128,2*TBL]:
    pair hp holds heads (2hp, 2hp+1) side by side along the free dim."""
    out = np.empty((2, KC, 2 * TBL), dtype=att_np_dt)
    for hp in range(2):
        out[hp, :, :TBL] = ebt_4[2 * hp]
        out[hp, :, TBL:] = ebt_4[2 * hp + 1]
    return out


# ------------------------------------------------------------- kernel body
def mha_body(tc, outs, ins, ckpt=None, dump=None):
    nc = tc.nc
    ctx = ExitStack()
    xt_d = ins["xt"]        # [1024, 2048] f16
    wq_d = ins["wq"]        # [1024, 256] f16
    wk_d = ins["wk"]        # [1024, 256] f16
    wv_d = ins["wv"]        # [1024, 256] f16
    wo_d = ins["wo"]        # [256, 1024] f16
    eb_d = ins["expb"]                    # [2, 128, 2*TBL] ATT_DT
    out_d = outs["out"]                   # [2048, 1024] f32

    att_np = ATT_DT

    with ctx:
        const = ctx.enter_context(tc.tile_pool(name="const", bufs=1))

        # ---- persistent SBUF tensors
        qt = [const.tile([128, S], F16, tag=f"qt{i}", name=f"qt{i}") for i in range(2)]
        kt = [const.tile([128, S], F16, tag=f"kt{i}", name=f"kt{i}") for i in range(2)]
        # V with a ones column per head: [k, 4*65]; bf16 (AV stationary)
        vsb = [const.tile([128, HPC * 65], att_np, tag=f"v{i}", name=f"v{i}")
               for i in range(NK)]
        # normalized attention outputs, head-pairs stacked on partitions
        ust = [const.tile([128, S], F16, tag=f"ust{i}", name=f"ust{i}") for i in range(2)]
        wo = [const.tile([128, D_MODEL], F16, tag=f"wo{i}", name=f"wo{i}") for i in range(2)]
        # pair-packed exp-bias tables, head j of pair hp at cols [j*TBL, (j+1)*TBL)
        ebp = [const.tile([128, 2 * TBL], att_np, tag=f"eb{i}", name=f"eb{i}")
               for i in range(2)]

        wpool = ctx.enter_context(tc.tile_pool(name="wqkv", bufs=1))
        wq = [wpool.tile([128, DH], F16, tag=f"wq{i}", name=f"wq{i}") for i in range(DKN)]
        wk = [wpool.tile([128, DH], F16, tag=f"wk{i}", name=f"wk{i}") for i in range(DKN)]
        wv = [wpool.tile([128, DH], F16, tag=f"wv{i}", name=f"wv{i}") for i in range(DKN)]

        # ---- flat working pools (no scoped release barriers)
        # 8 bufs: all of a q-block pair's [128,1024] xt tiles stay live
        xtp = ctx.enter_context(tc.tile_pool(name="xts", bufs=8))
        esp = ctx.enter_context(tc.tile_pool(name="es", bufs=4))
        esbp = ctx.enter_context(tc.tile_pool(name="esb", bufs=4))
        rzp = ctx.enter_context(tc.tile_pool(name="rz", bufs=2))
        outp = ctx.enter_context(tc.tile_pool(name="outsb", bufs=2))
        # PSUM: tag q [128,512]x2 (2 banks) for pk/pq/po, tag s [128,1024]x2
        # (4 banks) for pv/ps_pair, tag u [65,512]x2 (2 banks) for pus
        pp = ctx.enter_context(tc.tile_pool(name="pp", bufs=2, space="PSUM"))

        # ones columns of V (disjoint from the V copies below)
        for kc in range(NK):
            v3 = vsb[kc].rearrange("p (h c) -> p h c", h=HPC)
            nc.vector.memset(v3[:, :, 64:65], 1.0)

        # ================= pass 1: Q + K + V =================
        # One xt stream feeds all three projections per q block: Q/K use xt
        # as the moving operand, V uses its [128,128] column slices as the
        # stationary operand (pv[c] accumulates [k, dh] over d_model chunks).
        # All 8 PSUM banks are live per block: pq(u x2) pk(q x2) pv(a2+b2).
        # DMA issue: weights interleaved with qb=0's xt tiles, first-needed-
        # first; xt split across the SP and (idle) Pool queues; eb/wo late.
        # xt streamed as [128,1024] tiles, each covering two q blocks for one
        # d_model chunk (16 DMAs instead of 32)
        for qp in range(NQ // 2):
            xts = {}
            pvs = {}
            for qb in (2 * qp, 2 * qp + 1):
                pk = [pp.tile([128, QB], F32, tag="q", name=f"pk{m}_{qb}")
                      for m in range(2)]
                pq = [pp.tile([128, QB], F32, tag="u", name=f"pq{m}_{qb}")
                      for m in range(2)]
                # 4 k-chunk V accumulators [128,256]: one accumulation chain
                # per PSUM bank; chunks c, c+1 sit in the two banks of one
                # [128,1024] tile (cols 0:256 / 512:768)
                pvt = [pp.tile([128, 2 * QB], F32, tag="s",
                               name=f"pv{w}_{qb}") for w in range(2)]
                pv = [pvt[c // 2][:, (c % 2) * QB:(c % 2) * QB + DH]
                      for c in range(4)]
                pvs[qb] = pv
                for dk in range(DKN):
                    if qb == 0:
                        nc.sync.dma_start(
                            out=wk[dk], in_=wk_d[dk * 128:(dk + 1) * 128, :])
                        nc.sync.dma_start(
                            out=wv[dk], in_=wv_d[dk * 128:(dk + 1) * 128, :])
                        nc.gpsimd.dma_start(
                            out=wq[dk], in_=wq_d[dk * 128:(dk + 1) * 128, :])
                    if qb == 2 * qp:
                        # [128,1024] tile spans both blocks of the pair;
                        # queues alternate SP / Pool (halves DMA SEQ time)
                        eng = nc.gpsimd if dk % 2 else nc.sync
                        xtt = xtp.tile([128, 2 * QB], F16, tag="xt",
                                       name=f"xt_p1_{qp}_{dk}")
                        eng.dma_start(
                            out=xtt,
                            in_=xt_d[dk * 128:(dk + 1) * 128,
                                     qp * 2 * QB:(qp + 1) * 2 * QB])
                        xts[dk] = xtt
                    xtq = xts[dk][:, (qb % 2) * QB:(qb % 2 + 1) * QB]
                    for m in range(2):
                        nc.tensor.matmul(
                            pk[m], wk[dk][:, m * 128:(m + 1) * 128], xtq,
                            start=(dk == 0), stop=(dk == DKN - 1))
                    for m in range(2):
                        nc.tensor.matmul(
                            pq[m], wq[dk][:, m * 128:(m + 1) * 128], xtq,
                            start=(dk == 0), stop=(dk == DKN - 1))
                    for c in range(4):
                        nc.tensor.matmul(
                            pv[c], xtq[:, c * 128:(c + 1) * 128], wv[dk],
                            start=(dk == 0), stop=(dk == DKN - 1))
                if qb == 1:
                    # wo/eb transfers land in the pass-1 tail / early pass 2
                    for i in range(2):
                        nc.sync.dma_start(out=wo[i],
                                          in_=wo_d[i * 128:(i + 1) * 128, :])
                        nc.gpsimd.dma_start(out=ebp[i], in_=eb_d[i])
                # PSUM->SBUF copies on DVE (idle in pass 1; Act only exps)
                for m in range(2):
                    nc.vector.tensor_copy(
                        out=kt[m][:, qb * QB:(qb + 1) * QB], in_=pk[m])
                for m in range(2):
                    nc.vector.tensor_copy(
                        out=qt[m][:, qb * QB:(qb + 1) * QB], in_=pq[m])
                for c in range(4):
                    v3 = vsb[4 * qb + c].rearrange("p (h c) -> p h c", h=HPC)
                    nc.vector.tensor_copy(
                        out=v3[:, :, 0:64],
                        in_=pv[c].rearrange("p (h c) -> p h c", h=HPC))

        # ================= pass 2: per q block =================
        def emit_outproj_unit(qc, e):
            po = pp.tile([128, 512], F32, tag="q", name=f"po{qc}_{e}")
            for i in range(2):
                nc.tensor.matmul(
                    po,
                    ust[i][:, qc * 128:(qc + 1) * 128],
                    wo[i][:, e * 512:(e + 1) * 512],
                    start=(i == 0), stop=(i == 1))
            ob = outp.tile([128, 512], F32, tag="ob", name=f"ob{qc}_{e}")
            nc.vector.tensor_copy(out=ob, in_=po)
            nc.sync.dma_start(
                out=out_d[qc * 128:(qc + 1) * 128, e * 512:(e + 1) * 512],
                in_=ob)

        # software pipeline: out-proj(qb-1) units fill PE slack inside this
        # block's k loops (attention is Activation-bound).
        mulc = 0  # round-robin a quarter of the bias-muls onto idle Pool
        for qb in range(NQ):
            for hp in range(2):
                pus = [pp.tile([65, QB], F32, tag="u", name=f"pu{j}_{hp}_{qb}")
                       for j in range(2)]
                for kc in range(NK):
                    base = (TBL - S) - kc * 128 + qb * QB
                    # both heads' scores in one [128,1024] tile (one
                    # accumulation chain per bank), then ONE exp and ONE
                    # bias-multiply per kc (halves Act/DVE instruction count)
                    ps = pp.tile([128, 2 * QB], F32, tag="s",
                                 name=f"ps_{hp}_{qb}_{kc}")
                    for j in range(2):
                        prow = slice(j * 64, j * 64 + 64)
                        nc.tensor.matmul(
                            ps[:, j * QB:(j + 1) * QB],
                            kt[hp][prow, kc * 128:(kc + 1) * 128],
                            qt[hp][prow, qb * QB:(qb + 1) * QB],
                            start=True, stop=True)
                    es = esp.tile([128, 2 * QB], att_np, tag="es",
                                  name=f"es_{hp}_{qb}_{kc}")
                    nc.scalar.activation(out=es, in_=ps, func=AF.Exp)
                    esb = esbp.tile([128, 2 * QB], att_np, tag="esb",
                                    name=f"esb_{hp}_{qb}_{kc}")
                    ebv = ebp[hp].rearrange("p (j i) -> p j i", j=2)
                    nc.vector.tensor_mul(
                        esb.rearrange("p (j c) -> p j c", j=2),
                        es.rearrange("p (j c) -> p j c", j=2),
                        ebv[:, :, base:base + QB])
                    for j in range(2):
                        h = hp * 2 + j
                        nc.tensor.matmul(
                            pus[j], vsb[kc][:, h * 65:(h + 1) * 65],
                            esb[:, j * QB:(j + 1) * QB],
                            start=(kc == 0), stop=(kc == NK - 1))
                    # interleaved fill work for the PE: 8 out-proj units of
                    # block qb-1 spread over this block's 32 kc slots
                    if qb >= 1 and kc % 4 == 3:
                        u = hp * 4 + kc // 4
                        emit_outproj_unit((qb - 1) * 4 + u // 2, u % 2)
                # normalize U[d, q] / Z[q]; Z = row 64 of pus.  pus is copied
                # to SBUF first so the PSUM bank frees for the next head pair
                # without waiting out the whole normalize chain; 1/Z uses the
                # fast approximate reciprocal (51 ULP, 5x cheaper).
                for j in range(2):
                    pusb = rzp.tile([65, QB], F32, tag=f"pusb{j}",
                                    name=f"pusb{j}_{hp}_{qb}")
                    nc.vector.tensor_copy(out=pusb, in_=pus[j])
                    rz = rzp.tile([1, QB], F32, tag=f"rz{j}",
                                  name=f"rz{j}_{hp}_{qb}")
                    nc.vector.reciprocal(out=rz, in_=pusb[64:65, :])
                    rzb = rzp.tile([64, QB], F32, tag=f"rzb{j}",
                                   name=f"rzb{j}_{hp}_{qb}")
                    nc.gpsimd.partition_broadcast(rzb, rz, channels=64)
                    if j == 0:
                        nc.vector.tensor_mul(
                            ust[hp][0:64, qb * QB:(qb + 1) * QB],
                            pusb[0:64, :], rzb)
                    else:
                        # DVE lanes are partition-locked; write via a [64,512]
                        # staging tile then DMA to rows 64-127
                        stg = rzp.tile([64, QB], F16, tag="stg",
                                       name=f"stg{hp}_{qb}")
                        nc.vector.tensor_mul(stg, pusb[0:64, :], rzb)
                        nc.sync.dma_start(
                            out=ust[hp][64:128, qb * QB:(qb + 1) * QB],
                            in_=stg)

        for qc in range((NQ - 1) * 4, NQ * 4):
            for e in range(2):
                emit_outproj_unit(qc, e)

        if dump is not None:
            _, dbg = dump
            for i in range(2):
                pass  # dump disabled for f16
                pass
                pass
            for kc in range(NK):
                stg = outp.tile([128, HPC * 65], F32, tag="ob", name=f"dv{kc}")
                nc.vector.tensor_copy(out=stg, in_=vsb[kc])
                nc.sync.dma_start(out=dbg["d_vsb"][kc], in_=stg)


# ------------------------------------------------------------- build + run
def _declare_io(nc):
    """Declare the kernel's DRAM inputs/outputs on a Bacc instance."""
    ins = {
        "xt": nc.dram_tensor("xt", [D_MODEL, S], F16, kind="ExternalInput").ap(),
        "wq": nc.dram_tensor("wq", [D_MODEL, DH], F16, kind="ExternalInput").ap(),
        "wk": nc.dram_tensor("wk", [D_MODEL, DH], F16, kind="ExternalInput").ap(),
        "wv": nc.dram_tensor("wv", [D_MODEL, DH], F16, kind="ExternalInput").ap(),
        "wo": nc.dram_tensor("wo", [DH, D_MODEL], F16, kind="ExternalInput").ap(),
        "expb": nc.dram_tensor("expb", [2, KC, 2 * TBL], ATT_DT,
                               kind="ExternalInput").ap(),
    }
    outs = {
        "out": nc.dram_tensor("out", [S, D_MODEL], F32, kind="ExternalOutput").ap(),
    }
    return ins, outs


def _build():
    if "nc" in _cache:
        return _cache["nc"]
    nc = bacc.Bacc("TRN2", target_bir_lowering=False, debug=False)
    ins, outs = _declare_io(nc)
    with tile.TileContext(nc) as tc:
        mha_body(tc, outs, ins)
    nc.compile()
    _cache["nc"] = nc
    return nc


TRACE = False
LAST = {}


def _in_maps(inputs, Wq, Wk, Wv, Wo, rel_emb):
    att_np_dt = mybir.dt.np(ATT_DT)
    ebt = _expbias_tables(rel_emb)  # [16, 128, TBL] f32
    maps = []
    for c in range(N_CORES):
        b, g = c // (N_CORES // B), c % (N_CORES // B)
        hs = slice(g * DH, (g + 1) * DH)
        maps.append({
            "xt": np.ascontiguousarray(inputs[b].T).astype(np.float16),
            "wq": np.ascontiguousarray(Wq[:, hs]).astype(np.float16),
            "wk": np.ascontiguousarray(Wk[:, hs]).astype(np.float16),
            "wv": np.ascontiguousarray(Wv[:, hs]).astype(np.float16),
            "wo": np.ascontiguousarray(Wo[hs, :]).astype(np.float16),
            "expb": _pack_expb(ebt[g * HPC:(g + 1) * HPC], att_np_dt),
        })
    return maps


def kernel(inputs, Wq, Wk, Wv, Wo, rel_emb):
    inputs = np.asarray(inputs, dtype=np.float32)
    Wq = np.asarray(Wq, dtype=np.float32)
    Wk = np.asarray(Wk, dtype=np.float32)
    Wv = np.asarray(Wv, dtype=np.float32)
    Wo = np.asarray(Wo, dtype=np.float32)
    rel_emb = np.asarray(rel_emb, dtype=np.float32)

    nc = _build()
    in_maps = _in_maps(inputs, Wq, Wk, Wv, Wo, rel_emb)
    res = run_bass_kernel_spmd(
        nc, in_maps, core_ids=list(range(N_CORES)), trace=TRACE)
    LAST["res"] = res

    out = np.zeros((B, S, D_MODEL), dtype=np.float32)
    for c in range(N_CORES):
        b = c // (N_CORES // B)
        out[b] += res.results[c]["out"]
    return out
